# revision 9
# baseline (speedup 1.0000x reference)
"""Multi-head attention Bass/Tile kernel for 8 TRN2 NeuronCores.

Problem: nn_MultiHeadAttention (B=4, T1=T2=2048, d_model=256, d_key=32, H=8,
per-head value dim = d_model).  Reference math (no score scaling, no mask):

    k = key   @ WK^T + bk           [B, T1, 256]   (head h -> cols 32h..32h+32)
    q = query @ WQ^T + bq           [B, T2, 256]
    v = value @ WV^T + bv           [B, T1, 2048]  (head h -> cols 256h..256h+256)
    scores_h = k_h q_h^T            [T1, T2]
    attn = softmax over T1 (keys)
    emb_h = attn^T v_h              [T2, 256]
    out = emb' @ WO^T + bo          emb' channel c = d*8 + h (d outer, h inner)

Sharding: core c handles (batch b = c//2, query half qs = c%2) -> each core
computes the full output slice out[b, qs*1024:(qs+1)*1024, :].  No collectives.

Per-core algorithm (all matmuls bf16 with fp32 PSUM accumulation):
  - load activations + weights fp32, transpose via PE into channel-major
    bf16 copies (the PSUM->SBUF copy performs the cast)
  - kT = WKT^T keyT (+bk via ACT bias), qT likewise          [c, s] layouts
  - per head pair: v_pair = valueT^T WVT (+bv)               [s, c] natural
  - scores_h[s,q] = kT_h^T qT_h  (K=32 row-packed, 2 heads)  -> PSUM
  - E = exp(scores) via ACT (max|score| ~ 20, no max-subtraction needed),
    written straight to SBUF bf16
  - numerT_h[d,q] = v_h^T E  (PE, accumulated over s-tiles)
  - denom[q] = 1^T E (column-packed M=1 matmuls, 2 heads/slot)
  - out[q,:] = sum_h (numerT_h^T WOT'_h) * (1/denom_h[q]) + bo, where WOT' is
    WO column-permuted to head-outer so per-head rows are contiguous; the
    1/denom scale rides the per-partition scalar of scalar_tensor_tensor.

kernel(**inputs) takes the FULL unsharded inputs and returns the full output.
"""

import numpy as np
from contextlib import ExitStack

import concourse.bass as bass
import concourse.bacc as bacc
import concourse.mybir as mybir
import concourse.tile as tile
from concourse.bass_utils import run_bass_kernel_spmd
from concourse.masks import make_identity

P = 128
B, T1, T2, DM, DK, H = 4, 2048, 2048, 256, 32, 8
QSH = T2 // 2  # queries per core
N_CORES = 8

F32 = mybir.dt.float32
BF16 = mybir.dt.bfloat16
AF = mybir.ActivationFunctionType

ST = T1 // P        # 16 key/seq tiles
QT = QSH // P       # 8 query tiles per core
QC = 512            # query chunk (PSUM free dim)
NQC = QSH // QC     # 2 query chunks


def _load_chunked(nc, dst, src, n, chunks):
    """DMA src->dst split along the tile dim so consumers can start early."""
    step = max(1, n // chunks)
    for i in range(0, n, step):
        j = min(n, i + step)
        nc.sync.dma_start(out=dst[:, i:j, :], in_=src[:, i:j, :])


def _build_bass():
    nc = bacc.Bacc("TRN2", target_bir_lowering=False, debug=False)

    key = nc.dram_tensor("key_x", [T1, DM], F32, kind="ExternalInput").ap()
    qry = nc.dram_tensor("qry_x", [QSH, DM], F32, kind="ExternalInput").ap()
    val = nc.dram_tensor("val_x", [T1, DM], F32, kind="ExternalInput").ap()
    wk = nc.dram_tensor("wk", [DM, DM], F32, kind="ExternalInput").ap()
    wkb = nc.dram_tensor("wkb", [DM], F32, kind="ExternalInput").ap()
    wq = nc.dram_tensor("wq", [DM, DM], F32, kind="ExternalInput").ap()
    wqb = nc.dram_tensor("wqb", [DM], F32, kind="ExternalInput").ap()
    wv = nc.dram_tensor("wv", [H * DM, DM], F32, kind="ExternalInput").ap()
    wvb = nc.dram_tensor("wvb", [H * DM], F32, kind="ExternalInput").ap()
    wo = nc.dram_tensor("wo", [DM, H * DM], F32, kind="ExternalInput").ap()
    wob = nc.dram_tensor("wob", [DM], F32, kind="ExternalInput").ap()
    out = nc.dram_tensor("out_y", [QSH, DM], F32, kind="ExternalOutput").ap()

    with tile.TileContext(nc) as tc:
        with ExitStack() as ctx:
            _body(ctx, tc, key, qry, val, wk, wkb, wq, wqb, wv, wvb, wo, wob, out)
    nc.compile()
    return nc


def _body(ctx, tc, key, qry, val, wk, wkb, wq, wqb, wv, wvb, wo, wob, out):
    nc = tc.nc
    consts = ctx.enter_context(tc.tile_pool(name="consts", bufs=1))
    main = ctx.enter_context(tc.tile_pool(name="main", bufs=1))

    ident_f = consts.tile([P, P], F32)
    make_identity(nc, ident_f)
    ones_bf = consts.tile([P, 1], BF16)
    nc.vector.memset(ones_bf, 1.0)

    # biases; wk_b[p, t] = wkb[t*128+p] so kT tile ct gets bias wk_b[:, ct]
    wk_b = consts.tile([P, 2], F32)
    nc.sync.dma_start(out=wk_b, in_=wkb.rearrange("(t p) -> p t", p=P))
    wq_b = consts.tile([P, 2], F32)
    nc.sync.dma_start(out=wq_b, in_=wqb.rearrange("(t p) -> p t", p=P))
    # broadcast biases along partitions (step-0 partition AP)
    wvb_bc = consts.tile([P, H * DM], F32)
    nc.gpsimd.dma_start(
        out=wvb_bc,
        in_=bass.AP(tensor=wvb.tensor, offset=wvb.offset, ap=[[0, P], [1, H * DM]]),
    )
    wob_bc = consts.tile([P, DM], F32)
    nc.gpsimd.dma_start(
        out=wob_bc,
        in_=bass.AP(tensor=wob.tensor, offset=wob.offset, ap=[[0, P], [1, DM]]),
    )

    # channel-major bf16 tensors used by the main loop
    keyT = main.tile([P, 2, T1], BF16)    # [d, s]
    qryT = main.tile([P, 2, QSH], BF16)   # [d, q]
    valT = main.tile([P, 2, T1], BF16)    # [d, s]
    wkT = main.tile([P, 2, DM], BF16)     # [d, c]
    wqT = main.tile([P, 2, DM], BF16)
    wvT = main.tile([P, 2, H * DM], BF16)  # [d, c]
    woTp = main.tile([P, 16, DM], BF16)   # [c'=h*256+d, cout]
    kT = main.tile([P, 2, T1], BF16)      # [c, s]
    qT = main.tile([P, 2, QSH], BF16)     # [c, q]
    numerT = main.tile([P, 16, QSH], BF16)  # [c'=h*256+d, q] unnormalized
    recip = main.tile([P, H, QT], F32)    # [q%128, h, q//128] = 1/denom
    acc = main.tile([P, QT, DM], F32)     # output accumulator [q, cout]

    # ---------------- stage 0: load + transpose + k/q projections ----------
    with ExitStack() as s0:
        stg = s0.enter_context(tc.tile_pool(name="stg", bufs=1))
        pst = s0.enter_context(tc.tile_pool(name="pst", bufs=4, space="PSUM"))

        key_f = stg.tile([P, ST, DM], F32)
        _load_chunked(nc, key_f, key.rearrange("(n p) d -> p n d", p=P), ST, 4)
        qry_f = stg.tile([P, QT, DM], F32)
        _load_chunked(nc, qry_f, qry.rearrange("(n p) d -> p n d", p=P), QT, 2)
        val_f = stg.tile([P, ST, DM], F32)
        _load_chunked(nc, val_f, val.rearrange("(n p) d -> p n d", p=P), ST, 4)
        wk_f = stg.tile([P, 2, DM], F32)
        nc.sync.dma_start(out=wk_f, in_=wk.rearrange("(n p) d -> p n d", p=P))
        wq_f = stg.tile([P, 2, DM], F32)
        nc.sync.dma_start(out=wq_f, in_=wq.rearrange("(n p) d -> p n d", p=P))
        wv_f = stg.tile([P, ST, DM], F32)
        _load_chunked(nc, wv_f, wv.rearrange("(n p) d -> p n d", p=P), ST, 4)
        wo_f = stg.tile([P, 2, H * DM], F32)
        _load_chunked(nc, wo_f, wo.rearrange("(n p) d -> p n d", p=P), 2, 2)

        def tpose(dst, src, label):
            """dst[:, j, i*128:...] = transpose of 128x128 block src[:, i, j*128:...]"""
            pt = pst.tile([P, P], F32, tag="tp", name=f"tp_{label}", bufs=4)
            nc.tensor.transpose(pt, src, ident_f)
            nc.any.tensor_copy(out=dst, in_=pt)

        for st in range(ST):
            for dt in range(2):
                tpose(keyT[:, dt, st * P:(st + 1) * P],
                      key_f[:, st, dt * P:(dt + 1) * P], f"k{st}_{dt}")
                tpose(valT[:, dt, st * P:(st + 1) * P],
                      val_f[:, st, dt * P:(dt + 1) * P], f"v{st}_{dt}")
                tpose(wvT[:, dt, st * P:(st + 1) * P],
                      wv_f[:, st, dt * P:(dt + 1) * P], f"wv{st}_{dt}")
        for qt in range(QT):
            for dt in range(2):
                tpose(qryT[:, dt, qt * P:(qt + 1) * P],
                      qry_f[:, qt, dt * P:(dt + 1) * P], f"q{qt}_{dt}")
        for ct in range(2):
            for dt in range(2):
                tpose(wkT[:, dt, ct * P:(ct + 1) * P],
                      wk_f[:, ct, dt * P:(dt + 1) * P], f"wk{ct}_{dt}")
                tpose(wqT[:, dt, ct * P:(ct + 1) * P],
                      wq_f[:, ct, dt * P:(dt + 1) * P], f"wq{ct}_{dt}")
        # WO with head-outer column permutation: woTp row h*256+d = WO[:, d*8+h]
        wo_r = wo_f.rearrange("p t (d h) -> p t h d", h=H)  # [128, 2, 8, 256]
        for kt in range(16):
            h, db = kt // 2, kt % 2
            for ct in range(2):
                tpose(woTp[:, kt, ct * P:(ct + 1) * P],
                      wo_r[:, ct, h, db * P:(db + 1) * P], f"wo{kt}_{ct}")

        # k/q projections: kT[c, s] = sum_d wkT[d, c] keyT[d, s]  (+bias)
        for ct in range(2):
            for sc in range(T1 // 512):
                pp = pst.tile([P, 512], F32, tag="pp", name=f"ppk{ct}_{sc}", bufs=4)
                for dt in range(2):
                    nc.tensor.matmul(pp, wkT[:, dt, ct * P:(ct + 1) * P],
                                     keyT[:, dt, sc * 512:(sc + 1) * 512],
                                     start=(dt == 0), stop=(dt == 1))
                nc.scalar.activation(out=kT[:, ct, sc * 512:(sc + 1) * 512], in_=pp,
                                     func=AF.Identity, bias=wk_b[:, ct:ct + 1])
            for sc in range(QSH // 512):
                pp = pst.tile([P, 512], F32, tag="pp", name=f"ppq{ct}_{sc}", bufs=4)
                for dt in range(2):
                    nc.tensor.matmul(pp, wqT[:, dt, ct * P:(ct + 1) * P],
                                     qryT[:, dt, sc * 512:(sc + 1) * 512],
                                     start=(dt == 0), stop=(dt == 1))
                nc.scalar.activation(out=qT[:, ct, sc * 512:(sc + 1) * 512], in_=pp,
                                     func=AF.Identity, bias=wq_b[:, ct:ct + 1])

    # ---------------- main loop: attention per head pair --------------------
    with ExitStack() as sm:
        psc = sm.enter_context(tc.tile_pool(name="psc", bufs=2, space="PSUM"))
        pac = sm.enter_context(tc.tile_pool(name="pac", bufs=1, space="PSUM"))
        pdn = sm.enter_context(tc.tile_pool(name="pdn", bufs=1, space="PSUM"))
        pv = sm.enter_context(tc.tile_pool(name="pv", bufs=1, space="PSUM"))
        sE = sm.enter_context(tc.tile_pool(name="sE", bufs=3))
        sv = sm.enter_context(tc.tile_pool(name="sv", bufs=2))
        ssm = sm.enter_context(tc.tile_pool(name="ssm", bufs=2))

        for pg in range(H // 2):
            h0 = 2 * pg
            # v projection for this head pair: v_pair[s, 512] (heads h0, h0+1)
            v_pair = sv.tile([P, ST, 512], BF16, tag="vp", name=f"vp{pg}")
            for st in range(ST):
                pvt = pv.tile([P, 512], F32, tag="pv", name=f"pv{pg}_{st}")
                for dt in range(2):
                    nc.tensor.matmul(pvt, valT[:, dt, st * P:(st + 1) * P],
                                     wvT[:, dt, pg * 512:(pg + 1) * 512],
                                     start=(dt == 0), stop=(dt == 1))
                nc.vector.tensor_add(v_pair[:, st, :], pvt,
                                     wvb_bc[:, pg * 512:(pg + 1) * 512])

            for qc in range(NQC):
                Es = [sE.tile([P, ST, QC], BF16, tag="E", name=f"E{h0 + i}_{qc}")
                      for i in range(2)]
                # phase 1: scores + exp.  scores_h[s, q] = kT_h^T qT_h
                for sp in range(ST // 2):
                    for hh in range(2):
                        h = h0 + hh
                        base, ctile = 32 * (h % 4), h // 4
                        ps = psc.tile([P, 2, QC], F32, tag="sc",
                                      name=f"sc{h}_{qc}_{sp}")
                        for i in range(2):
                            st = 2 * sp + i
                            nc.tensor.matmul(
                                ps[:, i, :],
                                kT[base:base + 32, ctile, st * P:(st + 1) * P],
                                qT[base:base + 32, ctile, qc * QC:(qc + 1) * QC],
                                start=True, stop=True, tile_position=(base, 0))
                        nc.scalar.activation(out=Es[hh][:, 2 * sp:2 * sp + 2, :],
                                             in_=ps, func=AF.Exp)
                # phase 2: numerT_h[d, q] = v_h^T E_h ; denom = 1^T E_h
                for dh in range(2):
                    pas = [pac.tile([P, QC], F32, tag=f"acc{i}",
                                    name=f"pa{h0 + i}_{qc}_{dh}") for i in range(2)]
                    pd = None
                    if dh == 0:
                        pd = pdn.tile([P, QC], F32, tag="dn", name=f"pd{pg}_{qc}")
                    for st in range(ST):
                        for hh in range(2):
                            nc.tensor.matmul(
                                pas[hh],
                                v_pair[:, st, hh * 256 + dh * P: hh * 256 + (dh + 1) * P],
                                Es[hh][:, st, :],
                                start=(st == 0), stop=(st == ST - 1))
                        if dh == 0:
                            for hh in range(2):
                                nc.tensor.matmul(
                                    pd[32 * hh:32 * hh + 1, :], ones_bf,
                                    Es[hh][:, st, :],
                                    start=(st == 0), stop=(st == ST - 1),
                                    tile_position=(0, 32 * hh),
                                    skip_group_check=True)
                    for hh in range(2):
                        h = h0 + hh
                        nc.vector.tensor_copy(
                            out=numerT[:, 2 * h + dh, qc * QC:(qc + 1) * QC],
                            in_=pas[hh])
                    if dh == 0:
                        # denominators: copy out, transpose to [q, 1], invert.
                        # the transpose psum reuses the pdn pool slot (after
                        # pd is released), keeping total PSUM at 8 banks.
                        dsbs = []
                        for hh in range(2):
                            h = h0 + hh
                            dsb = ssm.tile([1, QC], F32, tag="dsb",
                                           name=f"dsb{h}_{qc}")
                            nc.vector.tensor_copy(out=dsb, in_=pd[32 * hh:32 * hh + 1, :])
                            dsbs.append(dsb)
                        pdt = pdn.tile([P, 2, QC // P], F32, tag="dn",
                                       name=f"pdt{pg}_{qc}")
                        for hh in range(2):
                            h = h0 + hh
                            for j in range(QC // P):
                                nc.tensor.transpose(
                                    pdt[:, hh, j:j + 1],
                                    dsbs[hh][:, j * P:(j + 1) * P],
                                    ident_f[0:1, 0:1])
                            nc.vector.reciprocal(
                                out=recip[:, h, qc * (QC // P):(qc + 1) * (QC // P)],
                                in_=pdt[:, hh, :])

    # ---------------- output: WO matmul with fused 1/denom ------------------
    with ExitStack() as sw:
        pw = sw.enter_context(tc.tile_pool(name="pw", bufs=8, space="PSUM"))

        mult, add = mybir.AluOpType.mult, mybir.AluOpType.add
        for qt in range(QT):
            for h in range(H):
                po = pw.tile([P, DM], F32, tag="po", name=f"po{qt}_{h}")
                for dh in range(2):
                    nc.tensor.matmul(po, numerT[:, 2 * h + dh, qt * P:(qt + 1) * P],
                                     woTp[:, 2 * h + dh, :],
                                     start=(dh == 0), stop=(dh == 1))
                nc.vector.scalar_tensor_tensor(
                    out=acc[:, qt, :], in0=po, scalar=recip[:, h, qt:qt + 1],
                    in1=(wob_bc if h == 0 else acc[:, qt, :]),
                    op0=mult, op1=add)
            nc.sync.dma_start(out=out.rearrange("(n p) d -> p n d", p=P)[:, qt, :],
                              in_=acc[:, qt, :])


_NC_CACHE = None


def _get_nc():
    global _NC_CACHE
    if _NC_CACHE is None:
        _NC_CACHE = _build_bass()
    return _NC_CACHE


def _make_in_maps(inputs):
    f = lambda x: np.ascontiguousarray(np.asarray(x, dtype=np.float32))
    shared = {
        "wk": f(inputs["WK_w"]), "wkb": f(inputs["WK_b"]),
        "wq": f(inputs["WQ_w"]), "wqb": f(inputs["WQ_b"]),
        "wv": f(inputs["WV_w"]), "wvb": f(inputs["WV_b"]),
        "wo": f(inputs["WO_w"]), "wob": f(inputs["WO_b"]),
    }
    key_in = f(inputs["key_input"])
    qry_in = f(inputs["query_input"])
    val_in = f(inputs["value_input"])
    in_maps = []
    for c in range(N_CORES):
        b, qs = c // 2, c % 2
        in_maps.append(dict(
            shared,
            key_x=np.ascontiguousarray(key_in[b]),
            qry_x=np.ascontiguousarray(qry_in[b, qs * QSH:(qs + 1) * QSH]),
            val_x=np.ascontiguousarray(val_in[b]),
        ))
    return in_maps


def _assemble(results):
    out = np.empty((B, T2, DM), dtype=np.float32)
    for c in range(N_CORES):
        b, qs = c // 2, c % 2
        out[b, qs * QSH:(qs + 1) * QSH] = results[c]["out_y"]
    return out


def run_spmd(inputs, **kwargs):
    """Run the kernel on all 8 cores; kwargs forwarded (e.g. trace=True)."""
    nc = _get_nc()
    res = run_bass_kernel_spmd(nc, _make_in_maps(inputs),
                               core_ids=list(range(N_CORES)), **kwargs)
    return res


def kernel(**inputs):
    res = run_spmd(inputs)
    return _assemble(res.results)
